# revision 4
# baseline (speedup 1.0000x reference)
"""TRN2 Bass kernel v2 for nn_HTModel: hierarchical-Tucker forward.

Data-parallel over batch (BC=512 rows/core). Per core, j-leaves are processed
in 16 groups of 4 (j0..j3), lane-paired: (j0,j1) data on partitions 0-63,
(j2,j3) on 64-127 via shifted (zero-padded) weight copies. X is
host-transposed so xT tiles are direct DMA loads. Layer-1 matmuls row-pack
two j's concurrently on the PE (disjoint K row-groups, separate PSUM banks);
layers 2/3/leaf merge the two lanes into ONE PSUM bank per pair via 2-term
full-K accumulation groups (zero-padded weights), halving evacuation volume.
PSUM evacuations are split ACT-vs-DVE per pair and batched per bank-pair.
Tree levels 1-2 complete within each group; levels 3+ use a binary counter
over groups, with tree emission software-pipelined one group behind the MLP.
All matmuls run in f32r (1 cycle/row at free >= 256).
"""
import sys

sys.path.insert(0, '/opt/trn_rl_repo')

import functools
import numpy as np
from contextlib import ExitStack

import concourse.bacc as bacc
import concourse.tile as tile
from concourse import mybir

F32 = mybir.dt.float32
F32R = mybir.dt.float32r
AFT = mybir.ActivationFunctionType
ADD = mybir.AluOpType.add
MAX = mybir.AluOpType.max

N_CORES = 8
B, NJ, S, M, Y = 4096, 64, 64, 32, 1000
RNK = [64, 128, 256, 512, 512, 512]
BC = B // N_CORES
NG = 16                       # groups of 4 leaves


def _body(nc, tc, T):
    ctx = ExitStack()
    with ctx:
        wp = ctx.enter_context(tc.tile_pool(name="wp", bufs=1))
        xp = ctx.enter_context(tc.tile_pool(name="xp", bufs=2))
        hp = ctx.enter_context(tc.tile_pool(name="hp", bufs=2))
        pp = ctx.enter_context(tc.tile_pool(name="pp", bufs=2))
        tp = ctx.enter_context(tc.tile_pool(name="tp", bufs=1))
        op = ctx.enter_context(tc.tile_pool(name="op", bufs=1))
        ps = ctx.enter_context(tc.tile_pool(name="ps", bufs=4, space="PSUM"))
        wsctx = ExitStack()
        ws = wsctx.enter_context(tc.tile_pool(name="ws", bufs=2))

        def dmaw(shape, key, dt=F32R):
            t = wp.tile(shape, dt, tag=key, name=key)
            nc.sync.dma_start(t[:], T[key][:])
            return t

        w1d = dmaw([128, 128], "w1d")
        w2 = dmaw([128, 128], "w2")
        w2h = dmaw([128, 128], "w2h")
        w3d = dmaw([128, 128], "w3d")
        w3h = dmaw([128, 128], "w3h")
        q0d = dmaw([128, 4096], "q0d")
        q0h = dmaw([128, 4096], "q0h")
        p1d = dmaw([128, 2048], "p1d")
        b1v = dmaw([128, 1], "b1v", F32)
        b2d = dmaw([128, 1], "b2d", F32)
        b3d = dmaw([128, 1], "b3d", F32)

        pend = {
            2: tp.tile([128, 1024], F32R, name="pend2"),
            3: tp.tile([128, 2048], F32R, name="pend3"),
            4: tp.tile([128, 2048], F32R, name="pend4"),
            5: tp.tile([128, 2048], F32R, name="pend5"),
        }
        pairT = tp.tile([128, 2048], F32R, name="pairT")

        def contract(blobs, f, rhs):
            """P_{f+1} contraction: rhs holds the level-f product (rank
            RNK[f], ktiles of 512 cols). `blobs` is one lhsT tile per
            otile-half, cols (p, oi) = p*256 + oi*128. Returns psum tiles
            (each [128, 1024] = 2 otiles) for rank RNK[f+1]."""
            kt = RNK[f] // 128 if RNK[f] >= 128 else 1
            outs = []
            for oh, blob in enumerate(blobs):
                o = ps.tile([128, 1024], F32, tag="ps", name=f"c{f}")
                for oi in range(2):
                    for p in range(kt):
                        nc.tensor.matmul(
                            o[:, oi * 512:(oi + 1) * 512],
                            blob[:, p * 256 + oi * 128:p * 256 + oi * 128 + 128],
                            rhs[:, p * 512:(p + 1) * 512],
                            start=(p == 0), stop=(p == kt - 1))
                outs.append(o)
            return outs

        p2w = None
        pw = {}
        p2ws = {}

        def mlp_part(g):
            xg = xp.tile([128, 1024], F32R, tag="xq", name="xq")
            nc.gpsimd.dma_start(xg[:], T["x"][g])
            if g % 4 == 0:
                p2w = ws.tile([128, 1024], F32R, tag="p2w", name="p2w")
                nc.sync.dma_start(
                    p2w[:], T["p2d"][:, (g // 4) * 1024:(g // 4 + 1) * 1024])
                p2ws[g // 4] = p2w
            # prefetch tree-fold weights this group will consume; one tile
            # per otile-half, each [128, kt*256]
            for f in (2, 3, 4):
                step = 2 ** (f - 1)
                if g % step == step - 1:
                    node = g // step
                    kt = RNK[f] // 128
                    hw_ = kt * 256
                    nh = (RNK[f + 1] // 128) // 2
                    tiles = []
                    for oh in range(nh):
                        wt = ws.tile([128, hw_], F32R, tag=f"p{f + 1}w{oh}",
                                     name=f"p{f + 1}w{oh}",
                                     bufs=(2 if f == 2 else 1))
                        nc.sync.dma_start(
                            wt[:],
                            T[f"p{f + 1}"][:, (node * nh + oh) * hw_:
                                           (node * nh + oh + 1) * hw_])
                        tiles.append(wt)
                    pw[f] = tiles

            # --- MLP layer 1: row-packed pairs (j0,j2), (j1,j3)
            ps_a = ps.tile([128, 1024], F32, tag="ps", name="ps_a")
            ps_b = ps.tile([128, 1024], F32, tag="ps", name="ps_b")
            for cb, pst in ((0, ps_a), (1, ps_b)):
                nc.tensor.matmul(pst[:, 0:512], w1d[0:64, :],
                                 xg[0:64, cb * 512:(cb + 1) * 512],
                                 start=True, stop=True)
                nc.tensor.matmul(pst[:, 512:1024], w1d[64:128, :],
                                 xg[64:128, cb * 512:(cb + 1) * 512],
                                 start=True, stop=True)
            h1a = hp.tile([128, 1024], F32R, tag="h1a", name="h1a")
            nc.scalar.activation(h1a[:], ps_a[:], AFT.Relu, bias=b1v[:])
            h1b = hp.tile([128, 1024], F32R, tag="h1b", name="h1b")
            nc.vector.tensor_scalar(h1b[:], ps_b[:], b1v[:], 0.0, ADD, MAX)

            # --- layer 2: K=128. Lane-merged banks: j-high (M=128 shifted,
            # writes all parts, zeros low) FIRST, then j-low (M=64)
            # overwrites parts 0-63. One bank per pair.
            ps_c = ps.tile([128, 512], F32, tag="ps", name="ps_c")
            ps_c2 = ps.tile([128, 512], F32, tag="ps", name="ps_c2")
            for pst, h1 in ((ps_c, h1a), (ps_c2, h1b)):
                nc.tensor.matmul(pst[:, :], w2h[:, :], h1[:, 512:1024],
                                 start=True, stop=False)
                nc.tensor.matmul(pst[:, :], w2[:, :], h1[:, 0:512],
                                 start=False, stop=True)
            h2a = hp.tile([128, 512], F32R, tag="h2a", name="h2a", bufs=2)
            nc.scalar.activation(h2a[:], ps_c[:], AFT.Relu, bias=b2d[:])
            h2b = hp.tile([128, 512], F32R, tag="h2b", name="h2b", bufs=2)
            nc.vector.tensor_scalar(h2b[:], ps_c2[:], b2d[:], 0.0, ADD, MAX)

            # --- layer 3 (aug): lane-merged banks, j-high shifted first
            ps_d = ps.tile([128, 512], F32, tag="ps", name="ps_d")
            ps_d2 = ps.tile([128, 512], F32, tag="ps", name="ps_d2")
            for pst, h2 in ((ps_d, h2a), (ps_d2, h2b)):
                nc.tensor.matmul(pst[:, :], w3h[:, :],
                                 h2[:, :], start=True, stop=False)
                nc.tensor.matmul(pst[:, :], w3d[:, :],
                                 h2[:, :], start=False, stop=True)
            h3a = hp.tile([128, 512], F32R, tag="h3a", name="h3a", bufs=2)
            nc.scalar.activation(h3a[:], ps_d[:], AFT.Relu, bias=b3d[:])
            h3b = hp.tile([128, 512], F32R, tag="h3b", name="h3b", bufs=2)
            nc.vector.tensor_scalar(h3b[:], ps_d2[:], b3d[:], 0.0, ADD, MAX)

            # --- leaf: lane-merged banks; ps_e1 = even siblings (j0 lo /
            # j2 hi), ps_e2 = odd siblings (j1 lo / j3 hi)
            ps_e1 = ps.tile([128, 512], F32, tag="ps", name="ps_e1")
            ps_e2 = ps.tile([128, 512], F32, tag="ps", name="ps_e2")
            for cb, h3, pst in ((0, h3a, ps_e1), (1, h3b, ps_e2)):
                n = 2 * g + cb
                nc.tensor.matmul(pst[:, :],
                                 q0h[:, n * 128:(n + 1) * 128],
                                 h3[:, :], start=True, stop=False)
                nc.tensor.matmul(pst[:, :],
                                 q0d[:, n * 128:(n + 1) * 128],
                                 h3[:, :], start=False, stop=True)
            pend0 = pp.tile([128, 512], F32R, tag="pend0", name="pend0")
            nc.scalar.copy(pend0[:], ps_e1[:])
            pr0 = pp.tile([128, 512], F32R, tag="pr0", name="pr0")
            nc.vector.tensor_mul(pr0[:], pend0[:], ps_e2[:])
            return pr0

        def tree_part(g, pr0):
            # --- level-1 fold: row-packed (node 2g on L0, 2g+1 on L1)
            ps_f = ps.tile([128, 1024], F32, tag="ps", name="ps_f")
            nc.tensor.matmul(ps_f[:, 0:512], p1d[0:64, g * 128:(g + 1) * 128],
                             pr0[0:64, :], start=True, stop=True)
            nc.tensor.matmul(ps_f[:, 512:1024],
                             p1d[64:128, g * 128:(g + 1) * 128],
                             pr0[64:128, :], start=True, stop=True)

            # --- level-2 fold (within group): product of sibling L1 nodes
            pend1 = pp.tile([128, 512], F32R, tag="pend1", name="pend1")
            nc.scalar.copy(pend1[:], ps_f[:, 0:512])
            prod1 = pp.tile([128, 512], F32R, tag="prod1", name="prod1")
            nc.vector.tensor_mul(prod1[:], pend1[:], ps_f[:, 512:1024])
            # level-2 node g, rank 256
            p2w = p2ws[g // 4]
            cur = contract([p2w[:, (g % 4) * 256:(g % 4 + 1) * 256]], 1, prod1)

            # --- levels 3..6: binary counter over groups
            node, f = g, 2
            while node % 2 == 1 and f <= 5:
                if f == 5:
                    prodf = pairT
                else:
                    prodf = pp.tile([128, 4 * RNK[f]], F32R, tag=f"prod{f}",
                                    name=f"prod{f}", bufs=1)
                for i, c in enumerate(cur):
                    nc.vector.tensor_mul(
                        prodf[:, i * 1024:(i + 1) * 1024],
                        pend[f][:, i * 1024:(i + 1) * 1024], c[:])
                node //= 2
                if f == 5:
                    f += 1
                    break
                cur = contract(pw[f], f, prodf)
                f += 1
            if f <= 5:
                for i, c in enumerate(cur):
                    nc.scalar.copy(pend[f][:, i * 1024:(i + 1) * 1024], c[:])

        # software pipeline: tree of group g emitted after MLP of group g+1
        prev = None
        for g in range(NG):
            pr = mlp_part(g)
            if prev is not None:
                tree_part(g - 1, prev)
            prev = pr
        tree_part(NG - 1, prev)

        # --- top: out[b, y] = sum_a pair[a, b] Ptop[y, a]; pair stationary
        wsctx.close()   # tree weights done; reuse their SBUF for Ptop
        ptp = ctx.enter_context(tc.tile_pool(name="ptp", bufs=1))
        ptop = ptp.tile([128, 4000], F32R, name="ptop")
        nc.sync.dma_start(ptop[:], T["ptopm"][:])
        for bt in range(4):
            outb = op.tile([128, 1000], F32, tag="outb", name="outb")
            for yh in range(2):
                pt_ps = ps.tile([128, 500], F32, tag="ps", name="top")
                for pt in range(4):
                    nc.tensor.matmul(
                        pt_ps[:],
                        pairT[:, pt * 512 + bt * 128:pt * 512 + bt * 128 + 128],
                        ptop[:, (pt * 2 + yh) * 500:(pt * 2 + yh + 1) * 500],
                        start=(pt == 0), stop=(pt == 3))
                nc.scalar.copy(outb[:, yh * 500:(yh + 1) * 500], pt_ps[:])
            nc.sync.dma_start(T["out"][bt * 128:(bt + 1) * 128, :], outb[:])


def declare_params(nc):
    T = {}
    T["x"] = nc.declare_dram_parameter("x", [NG, 128, 1024], F32R, isOutput=False)
    T["w1d"] = nc.declare_dram_parameter("w1d", [128, 128], F32R, isOutput=False)
    T["w2"] = nc.declare_dram_parameter("w2", [128, 128], F32R, isOutput=False)
    T["w2h"] = nc.declare_dram_parameter("w2h", [128, 128], F32R, isOutput=False)
    T["w3d"] = nc.declare_dram_parameter("w3d", [128, 128], F32R, isOutput=False)
    T["w3h"] = nc.declare_dram_parameter("w3h", [128, 128], F32R, isOutput=False)
    T["b1v"] = nc.declare_dram_parameter("b1v", [128, 1], F32, isOutput=False)
    T["b2d"] = nc.declare_dram_parameter("b2d", [128, 1], F32, isOutput=False)
    T["b3d"] = nc.declare_dram_parameter("b3d", [128, 1], F32, isOutput=False)
    T["q0d"] = nc.declare_dram_parameter("q0d", [128, 4096], F32R, isOutput=False)
    T["q0h"] = nc.declare_dram_parameter("q0h", [128, 4096], F32R, isOutput=False)
    T["p1d"] = nc.declare_dram_parameter("p1d", [128, 2048], F32R, isOutput=False)
    T["p2d"] = nc.declare_dram_parameter("p2d", [128, 4096], F32R, isOutput=False)
    T["p3"] = nc.declare_dram_parameter("p3", [128, 8192], F32R, isOutput=False)
    T["p4"] = nc.declare_dram_parameter("p4", [128, 8192], F32R, isOutput=False)
    T["p5"] = nc.declare_dram_parameter("p5", [128, 4096], F32R, isOutput=False)
    T["ptopm"] = nc.declare_dram_parameter("ptopm", [128, 4000], F32R, isOutput=False)
    T["out"] = nc.declare_dram_parameter("out", [BC, Y], F32, isOutput=True)
    return T


def build_nc(reps=1, loop_n=1):
    nc = bacc.Bacc()
    T = declare_params(nc)
    with tile.TileContext(nc) as tc:
        if loop_n == 1:
            for _ in range(reps):
                _body(nc, tc, T)
        else:
            with tc.For_i(0, loop_n, 1):
                _body(nc, tc, T)
    nc.compile()
    return nc


def _tree_blob(P):
    """P (nj, r_out, r_in) -> lhsT blob [128, nj*kt*no*128], col order
    (node, othalf, ktile, oi) so an othalf block is contiguous."""
    nj, r_out, r_in = P.shape
    kt, no = (r_in + 127) // 128, (r_out + 127) // 128
    nh = max(no // 2, 1)
    psz = min(128, r_in)
    W = np.transpose(P, (0, 2, 1)).astype(np.float64)      # (nj, r_in, r_out)
    W = W.reshape(nj, kt, psz, nh, no // nh, min(128, r_out))
    W = np.transpose(W, (2, 0, 3, 1, 4, 5)).reshape(psz, -1)
    out = np.zeros((128, W.shape[1]), np.float32)
    out[:psz] = W.astype(np.float32)
    return np.ascontiguousarray(out)


def prepack(inputs):
    f = {k: np.asarray(v, dtype=np.float64) for k, v in inputs.items()
         if k != "X"}
    bl = {}
    w1 = f["W1"].astype(np.float32)                         # (64, 128)
    bl["w1d"] = np.ascontiguousarray(np.vstack([w1, w1]))
    w2p = np.zeros((128, 128), np.float32)
    w2p[:, 0:64] = f["W2"]
    bl["w2"] = w2p
    w2h = np.zeros((128, 128), np.float32)
    w2h[:, 64:128] = f["W2"]
    bl["w2h"] = w2h
    w3d = np.zeros((128, 128), np.float32)
    w3d[0:64, 0:32] = f["W3"]
    bl["w3d"] = w3d
    w3h = np.zeros((128, 128), np.float32)
    w3h[64:128, 64:96] = f["W3"]
    bl["w3h"] = w3h
    bl["b1v"] = np.ascontiguousarray(
        f["b1"].reshape(128, 1).astype(np.float32))
    b2d = np.concatenate([f["b2"], f["b2"]]).reshape(128, 1)
    bl["b2d"] = np.ascontiguousarray(b2d.astype(np.float32))
    b3d = np.zeros((128, 1), np.float32)
    b3d[0:32, 0] = f["b3"]
    b3d[32, 0] = 1.0
    b3d[64:96, 0] = f["b3"]
    b3d[96, 0] = 1.0
    bl["b3d"] = b3d
    # leaf: Q0[j] = (W4 @ P0[j].T -> [33,64] with bias row 32)
    q0 = np.einsum("km,jam->jka", f["W4"], f["P0"])         # (64, 32, 64)
    c0 = np.einsum("jam,m->ja", f["P0"], f["b4"])           # (64, 64)
    q0a = np.concatenate([q0, c0[:, None, :]], axis=1)      # (64, 33, 64)
    # q0d col n*64: low-lane j = 4*(n//2) + (n%2); q0h col n*128 (shifted
    # M=128, right half): high-lane j = that + 2
    q0d = np.zeros((128, 4096), np.float64)
    q0h = np.zeros((128, 4096), np.float64)
    for n in range(32):
        jlo = 4 * (n // 2) + (n % 2)
        q0d[0:33, n * 128:n * 128 + 64] = q0a[jlo]
        q0h[64:97, n * 128 + 64:(n + 1) * 128] = q0a[jlo + 2]
    bl["q0d"] = np.ascontiguousarray(q0d.astype(np.float32))
    bl["q0h"] = np.ascontiguousarray(q0h.astype(np.float32))
    # p1d: node 2g (from pair j0,j1) low lane; node 2g+1 high lane
    P1 = f["P1"]                                            # (32, 128, 64)
    p1d = np.zeros((128, 2048), np.float64)
    for g in range(16):
        p1d[0:64, g * 128:(g + 1) * 128] = P1[2 * g].T
        p1d[64:128, g * 128:(g + 1) * 128] = P1[2 * g + 1].T
    bl["p1d"] = np.ascontiguousarray(p1d.astype(np.float32))
    bl["p2d"] = _tree_blob(np.asarray(inputs["P2"], np.float64))
    bl["p3"] = _tree_blob(np.asarray(inputs["P3"], np.float64))
    bl["p4"] = _tree_blob(np.asarray(inputs["P4"], np.float64))
    bl["p5"] = _tree_blob(np.asarray(inputs["P5"], np.float64))
    ptop = f["Ptop"]                                        # (1000, 512)
    A = ptop.T.reshape(4, 128, 2, 500)
    bl["ptopm"] = np.ascontiguousarray(
        np.transpose(A, (1, 0, 2, 3)).reshape(128, 4000).astype(np.float32))
    return bl


def pack_x(X):
    """X (B, 64, 64) -> per-core [NG, 128, 1024]: part = lane*64 + s,
    col = (j%2)*512 + b, lane = (j%4)//2, group = j//4."""
    Xr = np.asarray(X, np.float32).reshape(N_CORES, BC, NG, 2, 2, S)
    xt = Xr.transpose(0, 2, 3, 5, 4, 1)     # core, g, lane, s, cb, b
    return np.ascontiguousarray(xt.reshape(N_CORES, NG, 128, 1024))


@functools.lru_cache(maxsize=4)
def _cached_nc(reps=1, loop_n=1):
    return build_nc(reps, loop_n)


def kernel(**inputs):
    from concourse.bass_utils import run_bass_kernel_spmd
    nc = _cached_nc(1)
    bl = prepack(inputs)
    xt = pack_x(inputs["X"])
    in_maps = [dict(bl, x=xt[c]) for c in range(N_CORES)]
    res = run_bass_kernel_spmd(nc, in_maps, list(range(N_CORES)))
    return np.concatenate([res.results[c]["out"] for c in range(N_CORES)],
                          axis=0)


# revision 5
# speedup vs baseline: 1.2235x; 1.2235x over previous
"""TRN2 Bass kernel v2 for nn_HTModel: hierarchical-Tucker forward.

Data-parallel over batch (BC=512 rows/core). Per core, j-leaves are processed
in 16 groups of 4 (j0..j3), lane-paired: (j0,j1) on partitions 0-63, (j2,j3)
on partitions 64-127. X is host-transposed so SBUF tiles are direct DMA loads.
MLP matmuls are quadrant-packed (row/col tile_position via base partitions) so
two j's run concurrently on the PE; PSUM evacuations are batched multi-bank
ACT/DVE ops. Tree level 1 completes within each group; levels 2+ use a
binary-counter over groups. All matmuls in f32r (1 cyc/row at free>=256).
"""
import sys

sys.path.insert(0, '/opt/trn_rl_repo')

import functools
import numpy as np
from contextlib import ExitStack

import concourse.bacc as bacc
import concourse.tile as tile
from concourse import mybir

F32 = mybir.dt.float32
F32R = mybir.dt.float32r
AFT = mybir.ActivationFunctionType
ADD = mybir.AluOpType.add
MAX = mybir.AluOpType.max

N_CORES = 8
B, NJ, S, M, Y = 4096, 64, 64, 32, 1000
RNK = [64, 128, 256, 512, 512, 512]
BC = B // N_CORES
NG = 16                       # groups of 4 leaves


def _body(nc, tc, T):
    ctx = ExitStack()
    with ctx:
        wp = ctx.enter_context(tc.tile_pool(name="wp", bufs=1))
        xp = ctx.enter_context(tc.tile_pool(name="xp", bufs=2))
        hp = ctx.enter_context(tc.tile_pool(name="hp", bufs=2))
        pp = ctx.enter_context(tc.tile_pool(name="pp", bufs=2))
        tp = ctx.enter_context(tc.tile_pool(name="tp", bufs=1))
        op = ctx.enter_context(tc.tile_pool(name="op", bufs=1))
        ps = ctx.enter_context(tc.tile_pool(name="ps", bufs=4, space="PSUM"))
        wsctx = ExitStack()
        ws = wsctx.enter_context(tc.tile_pool(name="ws", bufs=2))

        def dmaw(shape, key, dt=F32R):
            t = wp.tile(shape, dt, tag=key, name=key)
            nc.sync.dma_start(t[:], T[key][:])
            return t

        w1d = dmaw([128, 128], "w1d")
        w2 = dmaw([128, 128], "w2")
        w2h = dmaw([128, 128], "w2h")
        w3d = dmaw([128, 128], "w3d")
        w3h = dmaw([128, 128], "w3h")
        q0d = dmaw([128, 4096], "q0d")
        q0h = dmaw([128, 4096], "q0h")
        p1d = dmaw([128, 2048], "p1d")
        b1v = dmaw([128, 1], "b1v", F32)
        b2d = dmaw([128, 1], "b2d", F32)
        b3d = dmaw([128, 1], "b3d", F32)

        pend = {
            2: tp.tile([128, 1024], F32R, name="pend2"),
            3: tp.tile([128, 2048], F32R, name="pend3"),
            4: tp.tile([128, 2048], F32R, name="pend4"),
            5: tp.tile([128, 2048], F32R, name="pend5"),
        }
        pairT = tp.tile([128, 2048], F32R, name="pairT")

        def contract(blobs, f, rhs):
            """P_{f+1} contraction: rhs holds the level-f product (rank
            RNK[f], ktiles of 512 cols). `blobs` is one lhsT tile per
            otile-half, cols (p, oi) = p*256 + oi*128. Returns psum tiles
            (each [128, 1024] = 2 otiles) for rank RNK[f+1]."""
            kt = RNK[f] // 128 if RNK[f] >= 128 else 1
            outs = []
            for oh, blob in enumerate(blobs):
                o = ps.tile([128, 1024], F32, tag="ps", name=f"c{f}")
                for oi in range(2):
                    for p in range(kt):
                        nc.tensor.matmul(
                            o[:, oi * 512:(oi + 1) * 512],
                            blob[:, p * 256 + oi * 128:p * 256 + oi * 128 + 128],
                            rhs[:, p * 512:(p + 1) * 512],
                            start=(p == 0), stop=(p == kt - 1))
                outs.append(o)
            return outs

        p2w = None
        pw = {}
        p2ws = {}

        def mlp_part(g):
            xg = xp.tile([128, 1024], F32R, tag="xq", name="xq")
            nc.gpsimd.dma_start(xg[:], T["x"][g])
            if g % 4 == 0:
                p2w = ws.tile([128, 1024], F32R, tag="p2w", name="p2w")
                nc.sync.dma_start(
                    p2w[:], T["p2d"][:, (g // 4) * 1024:(g // 4 + 1) * 1024])
                p2ws[g // 4] = p2w
            # prefetch tree-fold weights this group will consume; one tile
            # per otile-half, each [128, kt*256]
            for f in (2, 3, 4):
                step = 2 ** (f - 1)
                if g % step == step - 1:
                    node = g // step
                    kt = RNK[f] // 128
                    hw_ = kt * 256
                    nh = (RNK[f + 1] // 128) // 2
                    tiles = []
                    for oh in range(nh):
                        wt = ws.tile([128, hw_], F32R, tag=f"p{f + 1}w{oh}",
                                     name=f"p{f + 1}w{oh}",
                                     bufs=(2 if f == 2 else 1))
                        nc.sync.dma_start(
                            wt[:],
                            T[f"p{f + 1}"][:, (node * nh + oh) * hw_:
                                           (node * nh + oh + 1) * hw_])
                        tiles.append(wt)
                    pw[f] = tiles

            # --- MLP layer 1: row-packed pairs (j0,j2), (j1,j3)
            pa = [ps.tile([128, 512], F32, tag="ps", name=f"pa{i}")
                  for i in range(4)]
            for cb in (0, 1):
                nc.tensor.matmul(pa[2 * cb][:, :], w1d[0:64, :],
                                 xg[0:64, cb * 512:(cb + 1) * 512],
                                 start=True, stop=True)
                nc.tensor.matmul(pa[2 * cb + 1][:, :], w1d[64:128, :],
                                 xg[64:128, cb * 512:(cb + 1) * 512],
                                 start=True, stop=True)
            h1a = hp.tile([128, 1024], F32R, tag="h1a", name="h1a")
            nc.scalar.activation(h1a[:, 0:512], pa[0][:], AFT.Relu, bias=b1v[:])
            nc.vector.tensor_scalar(h1a[:, 512:1024], pa[1][:], b1v[:], 0.0,
                                    ADD, MAX)
            h1b = hp.tile([128, 1024], F32R, tag="h1b", name="h1b")
            nc.scalar.activation(h1b[:, 0:512], pa[2][:], AFT.Relu, bias=b1v[:])
            nc.vector.tensor_scalar(h1b[:, 512:1024], pa[3][:], b1v[:], 0.0,
                                    ADD, MAX)

            # --- layer 2: K=128. Lane-merged banks: j-high (M=128 shifted,
            # writes all parts, zeros low) FIRST, then j-low (M=64)
            # overwrites parts 0-63. One bank per pair.
            ps_c = ps.tile([128, 512], F32, tag="ps", name="ps_c")
            ps_c2 = ps.tile([128, 512], F32, tag="ps", name="ps_c2")
            for pst, h1 in ((ps_c, h1a), (ps_c2, h1b)):
                nc.tensor.matmul(pst[:, :], w2h[:, :], h1[:, 512:1024],
                                 start=True, stop=False)
                nc.tensor.matmul(pst[:, :], w2[:, :], h1[:, 0:512],
                                 start=False, stop=True)
            h2a = hp.tile([128, 512], F32R, tag="h2a", name="h2a", bufs=2)
            nc.scalar.activation(h2a[:], ps_c[:], AFT.Relu, bias=b2d[:])
            h2b = hp.tile([128, 512], F32R, tag="h2b", name="h2b", bufs=2)
            nc.vector.tensor_scalar(h2b[:], ps_c2[:], b2d[:], 0.0, ADD, MAX)

            # --- layer 3 (aug): lane-merged banks, j-high shifted first
            ps_d = ps.tile([128, 512], F32, tag="ps", name="ps_d")
            ps_d2 = ps.tile([128, 512], F32, tag="ps", name="ps_d2")
            for pst, h2 in ((ps_d, h2a), (ps_d2, h2b)):
                nc.tensor.matmul(pst[:, :], w3h[:, :],
                                 h2[:, :], start=True, stop=False)
                nc.tensor.matmul(pst[:, :], w3d[:, :],
                                 h2[:, :], start=False, stop=True)
            h3a = hp.tile([128, 512], F32R, tag="h3a", name="h3a", bufs=2)
            nc.scalar.activation(h3a[:], ps_d[:], AFT.Relu, bias=b3d[:])
            h3b = hp.tile([128, 512], F32R, tag="h3b", name="h3b", bufs=2)
            nc.vector.tensor_scalar(h3b[:], ps_d2[:], b3d[:], 0.0, ADD, MAX)

            # --- leaf: lane-merged banks; ps_e1 = even siblings (j0 lo /
            # j2 hi), ps_e2 = odd siblings (j1 lo / j3 hi)
            ps_e1 = ps.tile([128, 512], F32, tag="ps", name="ps_e1")
            ps_e2 = ps.tile([128, 512], F32, tag="ps", name="ps_e2")
            for cb, h3, pst in ((0, h3a, ps_e1), (1, h3b, ps_e2)):
                n = 2 * g + cb
                nc.tensor.matmul(pst[:, :],
                                 q0h[:, n * 128:(n + 1) * 128],
                                 h3[:, :], start=True, stop=False)
                nc.tensor.matmul(pst[:, :],
                                 q0d[:, n * 128:(n + 1) * 128],
                                 h3[:, :], start=False, stop=True)
            pend0 = pp.tile([128, 512], F32R, tag="pend0", name="pend0")
            nc.scalar.copy(pend0[:], ps_e1[:])
            pr0 = pp.tile([128, 512], F32R, tag="pr0", name="pr0")
            nc.vector.tensor_mul(pr0[:], pend0[:], ps_e2[:])
            return pr0

        def tree_part(g, pr0):
            # --- level-1 fold: row-packed (node 2g on L0, 2g+1 on L1)
            ps_f = ps.tile([128, 1024], F32, tag="ps", name="ps_f")
            nc.tensor.matmul(ps_f[:, 0:512], p1d[0:64, g * 128:(g + 1) * 128],
                             pr0[0:64, :], start=True, stop=True)
            nc.tensor.matmul(ps_f[:, 512:1024],
                             p1d[64:128, g * 128:(g + 1) * 128],
                             pr0[64:128, :], start=True, stop=True)

            # --- level-2 fold (within group): product of sibling L1 nodes
            pend1 = pp.tile([128, 512], F32R, tag="pend1", name="pend1")
            nc.scalar.copy(pend1[:], ps_f[:, 0:512])
            prod1 = pp.tile([128, 512], F32R, tag="prod1", name="prod1")
            nc.vector.tensor_mul(prod1[:], pend1[:], ps_f[:, 512:1024])
            # level-2 node g, rank 256
            p2w = p2ws[g // 4]
            cur = contract([p2w[:, (g % 4) * 256:(g % 4 + 1) * 256]], 1, prod1)

            # --- levels 3..6: binary counter over groups
            node, f = g, 2
            while node % 2 == 1 and f <= 5:
                if f == 5:
                    prodf = pairT
                else:
                    prodf = pp.tile([128, 4 * RNK[f]], F32R, tag=f"prod{f}",
                                    name=f"prod{f}", bufs=1)
                for i, c in enumerate(cur):
                    nc.vector.tensor_mul(
                        prodf[:, i * 1024:(i + 1) * 1024],
                        pend[f][:, i * 1024:(i + 1) * 1024], c[:])
                node //= 2
                if f == 5:
                    f += 1
                    break
                cur = contract(pw[f], f, prodf)
                f += 1
            if f <= 5:
                for i, c in enumerate(cur):
                    nc.scalar.copy(pend[f][:, i * 1024:(i + 1) * 1024], c[:])

        # software pipeline: tree of group g emitted after MLP of group g+1
        prev = None
        for g in range(NG):
            pr = mlp_part(g)
            if prev is not None:
                tree_part(g - 1, prev)
            prev = pr
        tree_part(NG - 1, prev)

        # --- top: out[b, y] = sum_a pair[a, b] Ptop[y, a]; pair stationary
        wsctx.close()   # tree weights done; reuse their SBUF for Ptop
        ptp = ctx.enter_context(tc.tile_pool(name="ptp", bufs=1))
        ptop = ptp.tile([128, 4000], F32R, name="ptop")
        nc.sync.dma_start(ptop[:], T["ptopm"][:])
        for bt in range(4):
            outb = op.tile([128, 1000], F32, tag="outb", name="outb")
            for yh in range(2):
                pt_ps = ps.tile([128, 500], F32, tag="ps", name="top")
                for pt in range(4):
                    nc.tensor.matmul(
                        pt_ps[:],
                        pairT[:, pt * 512 + bt * 128:pt * 512 + bt * 128 + 128],
                        ptop[:, (pt * 2 + yh) * 500:(pt * 2 + yh + 1) * 500],
                        start=(pt == 0), stop=(pt == 3))
                nc.scalar.copy(outb[:, yh * 500:(yh + 1) * 500], pt_ps[:])
            nc.sync.dma_start(T["out"][bt * 128:(bt + 1) * 128, :], outb[:])


def declare_params(nc):
    T = {}
    T["x"] = nc.declare_dram_parameter("x", [NG, 128, 1024], F32R, isOutput=False)
    T["w1d"] = nc.declare_dram_parameter("w1d", [128, 128], F32R, isOutput=False)
    T["w2"] = nc.declare_dram_parameter("w2", [128, 128], F32R, isOutput=False)
    T["w2h"] = nc.declare_dram_parameter("w2h", [128, 128], F32R, isOutput=False)
    T["w3d"] = nc.declare_dram_parameter("w3d", [128, 128], F32R, isOutput=False)
    T["w3h"] = nc.declare_dram_parameter("w3h", [128, 128], F32R, isOutput=False)
    T["b1v"] = nc.declare_dram_parameter("b1v", [128, 1], F32, isOutput=False)
    T["b2d"] = nc.declare_dram_parameter("b2d", [128, 1], F32, isOutput=False)
    T["b3d"] = nc.declare_dram_parameter("b3d", [128, 1], F32, isOutput=False)
    T["q0d"] = nc.declare_dram_parameter("q0d", [128, 4096], F32R, isOutput=False)
    T["q0h"] = nc.declare_dram_parameter("q0h", [128, 4096], F32R, isOutput=False)
    T["p1d"] = nc.declare_dram_parameter("p1d", [128, 2048], F32R, isOutput=False)
    T["p2d"] = nc.declare_dram_parameter("p2d", [128, 4096], F32R, isOutput=False)
    T["p3"] = nc.declare_dram_parameter("p3", [128, 8192], F32R, isOutput=False)
    T["p4"] = nc.declare_dram_parameter("p4", [128, 8192], F32R, isOutput=False)
    T["p5"] = nc.declare_dram_parameter("p5", [128, 4096], F32R, isOutput=False)
    T["ptopm"] = nc.declare_dram_parameter("ptopm", [128, 4000], F32R, isOutput=False)
    T["out"] = nc.declare_dram_parameter("out", [BC, Y], F32, isOutput=True)
    return T


def build_nc(reps=1, loop_n=1):
    nc = bacc.Bacc()
    T = declare_params(nc)
    with tile.TileContext(nc) as tc:
        if loop_n == 1:
            for _ in range(reps):
                _body(nc, tc, T)
        else:
            with tc.For_i(0, loop_n, 1):
                _body(nc, tc, T)
    nc.compile()
    return nc


def _tree_blob(P):
    """P (nj, r_out, r_in) -> lhsT blob [128, nj*kt*no*128], col order
    (node, othalf, ktile, oi) so an othalf block is contiguous."""
    nj, r_out, r_in = P.shape
    kt, no = (r_in + 127) // 128, (r_out + 127) // 128
    nh = max(no // 2, 1)
    psz = min(128, r_in)
    W = np.transpose(P, (0, 2, 1)).astype(np.float64)      # (nj, r_in, r_out)
    W = W.reshape(nj, kt, psz, nh, no // nh, min(128, r_out))
    W = np.transpose(W, (2, 0, 3, 1, 4, 5)).reshape(psz, -1)
    out = np.zeros((128, W.shape[1]), np.float32)
    out[:psz] = W.astype(np.float32)
    return np.ascontiguousarray(out)


def prepack(inputs):
    f = {k: np.asarray(v, dtype=np.float64) for k, v in inputs.items()
         if k != "X"}
    bl = {}
    w1 = f["W1"].astype(np.float32)                         # (64, 128)
    bl["w1d"] = np.ascontiguousarray(np.vstack([w1, w1]))
    w2p = np.zeros((128, 128), np.float32)
    w2p[:, 0:64] = f["W2"]
    bl["w2"] = w2p
    w2h = np.zeros((128, 128), np.float32)
    w2h[:, 64:128] = f["W2"]
    bl["w2h"] = w2h
    w3d = np.zeros((128, 128), np.float32)
    w3d[0:64, 0:32] = f["W3"]
    bl["w3d"] = w3d
    w3h = np.zeros((128, 128), np.float32)
    w3h[64:128, 64:96] = f["W3"]
    bl["w3h"] = w3h
    bl["b1v"] = np.ascontiguousarray(
        f["b1"].reshape(128, 1).astype(np.float32))
    b2d = np.concatenate([f["b2"], f["b2"]]).reshape(128, 1)
    bl["b2d"] = np.ascontiguousarray(b2d.astype(np.float32))
    b3d = np.zeros((128, 1), np.float32)
    b3d[0:32, 0] = f["b3"]
    b3d[32, 0] = 1.0
    b3d[64:96, 0] = f["b3"]
    b3d[96, 0] = 1.0
    bl["b3d"] = b3d
    # leaf: Q0[j] = (W4 @ P0[j].T -> [33,64] with bias row 32)
    q0 = np.einsum("km,jam->jka", f["W4"], f["P0"])         # (64, 32, 64)
    c0 = np.einsum("jam,m->ja", f["P0"], f["b4"])           # (64, 64)
    q0a = np.concatenate([q0, c0[:, None, :]], axis=1)      # (64, 33, 64)
    # q0d col n*64: low-lane j = 4*(n//2) + (n%2); q0h col n*128 (shifted
    # M=128, right half): high-lane j = that + 2
    q0d = np.zeros((128, 4096), np.float64)
    q0h = np.zeros((128, 4096), np.float64)
    for n in range(32):
        jlo = 4 * (n // 2) + (n % 2)
        q0d[0:33, n * 128:n * 128 + 64] = q0a[jlo]
        q0h[64:97, n * 128 + 64:(n + 1) * 128] = q0a[jlo + 2]
    bl["q0d"] = np.ascontiguousarray(q0d.astype(np.float32))
    bl["q0h"] = np.ascontiguousarray(q0h.astype(np.float32))
    # p1d: node 2g (from pair j0,j1) low lane; node 2g+1 high lane
    P1 = f["P1"]                                            # (32, 128, 64)
    p1d = np.zeros((128, 2048), np.float64)
    for g in range(16):
        p1d[0:64, g * 128:(g + 1) * 128] = P1[2 * g].T
        p1d[64:128, g * 128:(g + 1) * 128] = P1[2 * g + 1].T
    bl["p1d"] = np.ascontiguousarray(p1d.astype(np.float32))
    bl["p2d"] = _tree_blob(np.asarray(inputs["P2"], np.float64))
    bl["p3"] = _tree_blob(np.asarray(inputs["P3"], np.float64))
    bl["p4"] = _tree_blob(np.asarray(inputs["P4"], np.float64))
    bl["p5"] = _tree_blob(np.asarray(inputs["P5"], np.float64))
    ptop = f["Ptop"]                                        # (1000, 512)
    A = ptop.T.reshape(4, 128, 2, 500)
    bl["ptopm"] = np.ascontiguousarray(
        np.transpose(A, (1, 0, 2, 3)).reshape(128, 4000).astype(np.float32))
    return bl


def pack_x(X):
    """X (B, 64, 64) -> per-core [NG, 128, 1024]: part = lane*64 + s,
    col = (j%2)*512 + b, lane = (j%4)//2, group = j//4."""
    Xr = np.asarray(X, np.float32).reshape(N_CORES, BC, NG, 2, 2, S)
    xt = Xr.transpose(0, 2, 3, 5, 4, 1)     # core, g, lane, s, cb, b
    return np.ascontiguousarray(xt.reshape(N_CORES, NG, 128, 1024))


@functools.lru_cache(maxsize=4)
def _cached_nc(reps=1, loop_n=1):
    return build_nc(reps, loop_n)


def kernel(**inputs):
    from concourse.bass_utils import run_bass_kernel_spmd
    nc = _cached_nc(1)
    bl = prepack(inputs)
    xt = pack_x(inputs["X"])
    in_maps = [dict(bl, x=xt[c]) for c in range(N_CORES)]
    res = run_bass_kernel_spmd(nc, in_maps, list(range(N_CORES)))
    return np.concatenate([res.results[c]["out"] for c in range(N_CORES)],
                          axis=0)
